# revision 19
# baseline (speedup 1.0000x reference)
"""Trainium2 Bass kernel for nn_AuxLoss (aux CE loss + erf regularizer, segment-
mean over K=10 classes), data-parallel over 8 NeuronCores.

Math (per reference):
  f(u)      = 0.5 - 0.5*erf((-0.5 - u)/(sigma*sqrt2)) = 0.5 + 0.5*erf((u+0.5)*sqrt2)
  row_reg_n = sum_d f(u[n,d])
  row_ce_n  = logsumexp(yg[n,:]) - yg[n, yhat[n]]
  per-class means over rows with yhat==k, averaged over present classes:
  out = mean_k(seg_ce/cnt) + lmbd * mean_k(seg_reg/(cnt*D))

Device strategy per core (131072 rows = 128 partitions x 1024 contiguous rows):
  - DMA: sync (HWDGE) ring carries yg halves 0-1, yhat, u even chunks; gpsimd
    (SWDGE) ring carries yg halves 2-3 and u odd chunks. yg rides at the head
    of both rings so logsumexp is ready before the first work tile; the 16 SDMA
    engines stay saturated (~340 GB/s) start to finish.
  - ACT does exactly 3 table loads: exp over the 4 yg quarters (bf16 out),
    one ln over the whole [128,1024] sumexp, then 16 uninterrupted erf chunks
    tracking the u stream (explicit chain deps keep the order).
  - per u chunk one bf16 work tile [128, 64, 76]:
      cols 0:64  erf(sqrt2*u + sqrt2/2)  (the 0.5+0.5* affine is folded into
                 the host fixup: seg_f = 0.5*D*cnt + 0.5*seg_erf)
      cols 64:74 onehot*yg   (diagonal trick: row-sum of the segment matmul
                 block = seg of yg[n,yhat[n]])
      col  74    ones        (counts)
      col  75    lse         (segment-sum of logsumexp via the same matmul)
    onehot[p,r,c] = (yhat==c) via iota compare (DVE, bf16)
  - PE: per 128-row group one matmul: onehot[:,g,:] stationary (128x10),
    work[:,g,:] moving (128x76), accumulating PSUM [10,76] over 1024 groups.
  - Each core DMAs its raw [10,76] PSUM partials out (3 KB); the host gathers
    the 8 partials and finishes the ~40-flop reduction in float64. No
    collectives, no on-device epilogue.
"""

import math
import sys

if "/opt/trn_rl_repo" not in sys.path:
    sys.path.insert(0, "/opt/trn_rl_repo")

import numpy as np

N_CORES = 8
N_FULL = 1048576
C = 10
D = 64
P = 128
ROWS_PER_CORE = N_FULL // N_CORES  # 131072
RPP = ROWS_PER_CORE // P  # 1024 rows per partition
SQ2 = math.sqrt(2.0)
W = 64  # u rows/partition per chunk
NCH = RPP // W  # 16
U_BUFS = 6
WORK_BUFS = 3
YW = 256  # yg rows/partition per exp slice (4 slices of the 2 halves)
NYG = RPP // YW  # 4
W_COLS = D + C + 2  # erf block | onehot*yg block | ones | lse


def build():
    from concourse import bacc, mybir, tile
    from concourse.tile_rust import add_dep_helper

    f32 = mybir.dt.float32
    bf16 = mybir.dt.bfloat16
    i32 = mybir.dt.int32
    FT = mybir.ActivationFunctionType
    ALU = mybir.AluOpType
    AX = mybir.AxisListType

    nc = bacc.Bacc(
        "TRN2", target_bir_lowering=False, debug=False, num_devices=N_CORES
    )

    yh_d = nc.dram_tensor("yhat", [ROWS_PER_CORE], i32, kind="ExternalInput")
    yg_d = nc.dram_tensor("yg", [ROWS_PER_CORE, C], f32, kind="ExternalInput")
    u_d = nc.dram_tensor("u_zg", [ROWS_PER_CORE, D], f32, kind="ExternalInput")
    out_d = nc.dram_tensor("out", [C, W_COLS], f32, kind="ExternalOutput")

    u_v = u_d[:].rearrange("(p r) d -> p r d", p=P)
    yg_v = yg_d[:].rearrange("(p r) c -> p r c", p=P)
    yh_v = yh_d[:].rearrange("(p r) -> p r", p=P)

    last_act = [None]

    def act_ordered(*args, **kwargs):
        """scalar.activation with an explicit chain dep so the Tile scheduler
        cannot interleave ACT functions (each interleave costs a ~1.3 us
        ACT table-set load)."""
        inst = nc.scalar.activation(*args, **kwargs)
        raw = getattr(inst, "ins", inst)
        if last_act[0] is not None:
            add_dep_helper(raw, last_act[0], sync=True, reason="act set order")
        last_act[0] = raw
        return inst

    def chain(track, inst, why):
        """Force same-ring DMA trigger order (ring drains FIFO per engine)."""
        raw = getattr(inst, "ins", inst)
        if track[0] is not None:
            add_dep_helper(raw, track[0], sync=True, reason=why)
        track[0] = raw
        return inst

    last_sync = [None]
    last_gps = [None]

    with tile.TileContext(nc) as tc:
        with (
            tc.tile_pool(name="const", bufs=1) as constp,
            tc.tile_pool(name="io", bufs=1) as iop,
            tc.tile_pool(name="work", bufs=1) as workp,
            tc.tile_pool(name="psum", bufs=1, space="PSUM") as psump,
        ):
            # --- constants ---
            erf_bias = constp.tile([P, 1], f32)
            nc.vector.memset(erf_bias[:], 0.5 * SQ2)

            # --- DMA program ---
            # DMA ring entries do not pipeline their ~2us completion receipts,
            # so yg goes as ONE big transfer per ring (halves), then u chunks:
            # sync: ygH0, yhat, u evens; gpsimd: ygH1, u odds. Both rings are
            # pure-yg until ~13us, so exp/ln complete by ~23us and erf tracks
            # the u stream from its first chunk.
            yg_all = constp.tile([P, RPP, C], f32)
            HR = RPP // 2
            # sync head: a small first slice (so the first exp can start ~7us,
            # hiding the per-transfer completion-receipt latency) + the rest
            GB = RPP - YW  # gpsimd yg piece starts here (small: SWDGE starts late)
            chain(last_sync, nc.sync.dma_start(
                yg_all[:, 0:YW // 2, :], yg_v[:, 0:YW // 2, :]), "syncq")
            chain(last_sync, nc.sync.dma_start(
                yg_all[:, YW // 2:GB, :], yg_v[:, YW // 2:GB, :]), "syncq")
            yh_i = constp.tile([P, RPP], i32)
            chain(last_sync, nc.sync.dma_start(yh_i[:], yh_v), "syncq")
            chain(last_gps, nc.gpsimd.dma_start(
                yg_all[:, GB:RPP, :], yg_v[:, GB:RPP, :]), "gpsq")
            iota_f = constp.tile([P, 1, C], f32)
            nc.gpsimd.iota(
                iota_f[:, 0, :], [[1, C]],
                channel_multiplier=0, allow_small_or_imprecise_dtypes=True,
            )

            yh_f = constp.tile([P, RPP], f32)
            nc.vector.tensor_copy(yh_f[:], yh_i[:])

            u_ts = {}
            for ci in range(NCH):
                u_t = iop.tile([P, W, D], f32, name="u_t", bufs=U_BUFS)
                if ci % 2 == 0:
                    chain(last_sync,
                          nc.sync.dma_start(u_t[:], u_v[:, ci * W:(ci + 1) * W, :]),
                          "syncq")
                else:
                    chain(last_gps,
                          nc.gpsimd.dma_start(u_t[:], u_v[:, ci * W:(ci + 1) * W, :]),
                          "gpsq")
                u_ts[ci] = u_t

            # --- ACT: exp slices -> ln -> erf chunks (3 table loads total) ---
            # exp runs over SLICES aligned to the yg transfers so it pipelines
            # with their arrival; ex bufs=2 so exp never waits on a reduce.
            sume = constp.tile([P, RPP], f32)
            exp_slices = [(0, YW // 2), (YW // 2, GB), (GB, RPP)]
            for r0, r1 in exp_slices:
                ex_t = workp.tile([P, r1 - r0, C], bf16,
                                  name=f"ex_{r1 - r0}", bufs=1)
                act_ordered(ex_t[:], yg_all[:, r0:r1, :], FT.Exp)
                nc.vector.reduce_sum(sume[:, r0:r1], ex_t[:], axis=AX.X)
            lse = sume  # ln in place: sume becomes logsumexp
            act_ordered(lse[:], sume[:], FT.Ln)

            # --- streamed erf + work-tile assembly + segment matmuls ---
            ps = psump.tile([C, W_COLS], f32)
            for ci in range(NCH):
                r0, r1 = ci * W, (ci + 1) * W
                work_t = workp.tile([P, W, W_COLS], bf16, name="work_t", bufs=WORK_BUFS)
                act_ordered(
                    work_t[:, :, 0:D], u_ts.pop(ci)[:], FT.Erf,
                    bias=erf_bias[:], scale=SQ2,
                )
                oh_t = workp.tile([P, W, C], bf16, name="oh_t", bufs=WORK_BUFS)
                nc.vector.tensor_tensor(
                    oh_t[:],
                    yh_f[:, r0:r1].broadcast_to([P, W, C]),
                    iota_f[:].broadcast_to([P, W, C]),
                    ALU.is_equal,
                )
                nc.vector.tensor_tensor(
                    work_t[:, :, D:D + C], oh_t[:], yg_all[:, r0:r1, :], ALU.mult
                )
                nc.vector.memset(work_t[:, :, D + C], 1.0)
                nc.vector.tensor_copy(work_t[:, :, D + C + 1], lse[:, r0:r1])
                for g in range(W):
                    nc.tensor.matmul(
                        ps[:], oh_t[:, g, :], work_t[:, g, :],
                        start=(ci == 0 and g == 0),
                        stop=(ci == NCH - 1 and g == W - 1),
                    )

            # --- raw partials out; host finishes the 40-flop epilogue ---
            # out rides the (otherwise idle) scalar HWDGE ring, skipping the
            # sync ring's FIFO tail
            acc = constp.tile([C, W_COLS], f32)
            nc.vector.tensor_copy(acc[:], ps[:])
            out_trig = nc.scalar.dma_start(out_d[:], acc[:])
            add_dep_helper(getattr(out_trig, "ins", out_trig), last_act[0],
                           sync=True, reason="out trigger after last erf")

    nc.compile()
    return nc


_NC_CACHE = {}


def _get_nc():
    if "nc" not in _NC_CACHE:
        _NC_CACHE["nc"] = build()
    return _NC_CACHE["nc"]


def make_in_maps(yhat, yg, u_zg):
    yhat = np.ascontiguousarray(np.asarray(yhat).astype(np.int32))
    yg = np.ascontiguousarray(np.asarray(yg, dtype=np.float32))
    u_zg = np.ascontiguousarray(np.asarray(u_zg, dtype=np.float32))
    n = yhat.shape[0]
    assert n == ROWS_PER_CORE * N_CORES
    in_maps = []
    for i in range(N_CORES):
        s = slice(i * ROWS_PER_CORE, (i + 1) * ROWS_PER_CORE)
        in_maps.append({"yhat": yhat[s], "yg": yg[s], "u_zg": u_zg[s]})
    return in_maps


def _finalize(parts, lmbd):
    """Gather the 8 per-core [10,76] partials and finish in float64.
    cols 0:64 seg erf | 64:74 seg onehot*yg | 74 counts | 75 seg lse."""
    s = np.zeros((C, W_COLS), dtype=np.float64)
    for p in parts:
        s += np.asarray(p, dtype=np.float64)
    seg_erf = s[:, 0:D].sum(axis=1)
    picked = s[:, D:D + C].sum(axis=1)
    cnt = s[:, D + C]
    seg_lse = s[:, D + C + 1]
    present = cnt > 0
    denom = np.where(present, cnt, 1.0)
    # f(u) = 0.5 + 0.5*erf((u+0.5)*sqrt2): seg_f = 0.5*D*cnt + 0.5*seg_erf
    reg_c = (0.5 * D * cnt + 0.5 * seg_erf) / (denom * D)
    aux_c = (seg_lse - picked) / denom
    nuq = present.sum()
    val = (
        np.where(present, aux_c, 0.0).sum()
        + float(np.asarray(lmbd).reshape(())) * np.where(present, reg_c, 0.0).sum()
    ) / nuq
    return np.float32(val)


def run(yhat, yg, u_zg, lmbd, trace=False):
    from concourse import bass_utils

    nc = _get_nc()
    in_maps = make_in_maps(yhat, yg, u_zg)
    res = bass_utils.run_bass_kernel_spmd(
        nc, in_maps, core_ids=list(range(N_CORES)), trace=trace
    )
    parts = [res.results[i]["out"] for i in range(N_CORES)]
    return _finalize(parts, lmbd), res


def kernel(yhat, yg, u_zg, lmbd):
    val, _ = run(yhat, yg, u_zg, lmbd)
    return np.asarray(val, dtype=np.float32).reshape(())


# revision 20
# speedup vs baseline: 1.1140x; 1.1140x over previous
"""Trainium2 Bass kernel for nn_AuxLoss (aux CE loss + erf regularizer, segment-
mean over K=10 classes), data-parallel over 8 NeuronCores.

Math (per reference):
  f(u)      = 0.5 - 0.5*erf((-0.5 - u)/(sigma*sqrt2)) = 0.5 + 0.5*erf((u+0.5)*sqrt2)
  row_reg_n = sum_d f(u[n,d])
  row_ce_n  = logsumexp(yg[n,:]) - yg[n, yhat[n]]
  per-class means over rows with yhat==k, averaged over present classes:
  out = mean_k(seg_ce/cnt) + lmbd * mean_k(seg_reg/(cnt*D))

Device strategy per core (131072 rows = 128 partitions x 1024 contiguous rows):
  - DMA: sync (HWDGE) ring carries yg halves 0-1, yhat, u even chunks; gpsimd
    (SWDGE) ring carries yg halves 2-3 and u odd chunks. yg rides at the head
    of both rings so logsumexp is ready before the first work tile; the 16 SDMA
    engines stay saturated (~340 GB/s) start to finish.
  - ACT does exactly 3 table loads: exp over the 4 yg quarters (bf16 out),
    one ln over the whole [128,1024] sumexp, then 16 uninterrupted erf chunks
    tracking the u stream (explicit chain deps keep the order).
  - per u chunk one bf16 work tile [128, 64, 76]:
      cols 0:64  erf(sqrt2*u + sqrt2/2)  (the 0.5+0.5* affine is folded into
                 the host fixup: seg_f = 0.5*D*cnt + 0.5*seg_erf)
      cols 64:74 onehot*yg   (diagonal trick: row-sum of the segment matmul
                 block = seg of yg[n,yhat[n]])
      col  74    ones        (counts)
      col  75    lse         (segment-sum of logsumexp via the same matmul)
    onehot[p,r,c] = (yhat==c) via iota compare (DVE, bf16)
  - PE: per 128-row group one matmul: onehot[:,g,:] stationary (128x10),
    work[:,g,:] moving (128x76), accumulating PSUM [10,76] over 1024 groups.
  - Each core DMAs its raw [10,76] PSUM partials out (3 KB); the host gathers
    the 8 partials and finishes the ~40-flop reduction in float64. No
    collectives, no on-device epilogue.
"""

import math
import sys

if "/opt/trn_rl_repo" not in sys.path:
    sys.path.insert(0, "/opt/trn_rl_repo")

import numpy as np

N_CORES = 8
N_FULL = 1048576
C = 10
D = 64
P = 128
ROWS_PER_CORE = N_FULL // N_CORES  # 131072
RPP = ROWS_PER_CORE // P  # 1024 rows per partition
SQ2 = math.sqrt(2.0)
W = 64  # u rows/partition per chunk
NCH = RPP // W  # 16
U_BUFS = 6
WORK_BUFS = 3
YW = 256  # yg rows/partition per exp slice (4 slices of the 2 halves)
NYG = RPP // YW  # 4
W_COLS = D + C + 2  # erf block | onehot*yg block | ones | lse


def build():
    from concourse import bacc, mybir, tile
    from concourse.tile_rust import add_dep_helper

    f32 = mybir.dt.float32
    bf16 = mybir.dt.bfloat16
    i32 = mybir.dt.int32
    FT = mybir.ActivationFunctionType
    ALU = mybir.AluOpType
    AX = mybir.AxisListType

    nc = bacc.Bacc(
        "TRN2", target_bir_lowering=False, debug=False, num_devices=N_CORES
    )

    yh_d = nc.dram_tensor("yhat", [ROWS_PER_CORE], i32, kind="ExternalInput")
    yg_d = nc.dram_tensor("yg", [ROWS_PER_CORE, C], f32, kind="ExternalInput")
    u_d = nc.dram_tensor("u_zg", [ROWS_PER_CORE, D], f32, kind="ExternalInput")
    out_d = nc.dram_tensor("out", [C, W_COLS], f32, kind="ExternalOutput")

    u_v = u_d[:].rearrange("(p r) d -> p r d", p=P)
    yg_v = yg_d[:].rearrange("(p r) c -> p r c", p=P)
    yh_v = yh_d[:].rearrange("(p r) -> p r", p=P)

    last_act = [None]

    def act_ordered(*args, **kwargs):
        """scalar.activation with an explicit chain dep so the Tile scheduler
        cannot interleave ACT functions (each interleave costs a ~1.3 us
        ACT table-set load)."""
        inst = nc.scalar.activation(*args, **kwargs)
        raw = getattr(inst, "ins", inst)
        if last_act[0] is not None:
            add_dep_helper(raw, last_act[0], sync=True, reason="act set order")
        last_act[0] = raw
        return inst

    def chain(track, inst, why):
        """Force same-ring DMA trigger order (ring drains FIFO per engine)."""
        raw = getattr(inst, "ins", inst)
        if track[0] is not None:
            add_dep_helper(raw, track[0], sync=True, reason=why)
        track[0] = raw
        return inst

    last_sync = [None]
    last_gps = [None]

    with tile.TileContext(nc) as tc:
        with (
            tc.tile_pool(name="const", bufs=1) as constp,
            tc.tile_pool(name="io", bufs=1) as iop,
            tc.tile_pool(name="work", bufs=1) as workp,
            tc.tile_pool(name="psum", bufs=1, space="PSUM") as psump,
        ):
            # --- constants ---
            erf_bias = constp.tile([P, 1], f32)
            nc.vector.memset(erf_bias[:], 0.5 * SQ2)

            # --- DMA program ---
            # DMA ring entries do not pipeline their ~2us completion receipts,
            # so yg goes as ONE big transfer per ring (halves), then u chunks:
            # sync: ygH0, yhat, u evens; gpsimd: ygH1, u odds. Both rings are
            # pure-yg until ~13us, so exp/ln complete by ~23us and erf tracks
            # the u stream from its first chunk.
            # yg is split across all three rings (scalar+sync are fast-start
            # HWDGE; gpsimd's piece is small because SWDGE starts ~10us late),
            # and every ring's first u trigger is held behind tiny DVE gate
            # ops that read the other rings' yg slices -- concurrent bulk u
            # would starve yg (SWDGE wins arbitration). The scalar ring also
            # carries two tail u chunks whose triggers sit in the ACT chain
            # right after the erf that frees their pool slot (zero-wait).
            yg_all = constp.tile([P, RPP, C], f32)
            A1 = RPP // 2        # scalar ring: yg [0:A1]
            B1 = RPP - YW // 2   # sync: yg [A1:B1]; gpsimd: [B1:RPP]
            nc.scalar.dma_start(yg_all[:, 0:A1, :], yg_v[:, 0:A1, :])
            chain(last_sync, nc.sync.dma_start(
                yg_all[:, A1:B1, :], yg_v[:, A1:B1, :]), "syncq")
            yh_i = constp.tile([P, RPP], i32)
            chain(last_sync, nc.sync.dma_start(yh_i[:], yh_v), "syncq")
            chain(last_gps, nc.gpsimd.dma_start(
                yg_all[:, B1:RPP, :], yg_v[:, B1:RPP, :]), "gpsq")
            iota_f = constp.tile([P, 1, C], f32)
            nc.gpsimd.iota(
                iota_f[:, 0, :], [[1, C]],
                channel_multiplier=0, allow_small_or_imprecise_dtypes=True,
            )

            yh_f = constp.tile([P, RPP], f32)
            nc.vector.tensor_copy(yh_f[:], yh_i[:])

            # cross-ring yg-completion gates (deps are instruction-granular)
            gA = constp.tile([1, 3], f32)
            g_a = nc.vector.tensor_copy(gA[:, 0:1], yg_all[0:1, 1, 0:1])
            g_b = nc.vector.tensor_copy(gA[:, 1:2], yg_all[0:1, B1 - 1, 0:1])
            g_c = nc.vector.tensor_copy(gA[:, 2:3], yg_all[0:1, RPP - 1, 0:1])

            def gate(inst, *gs):
                for g in gs:
                    add_dep_helper(getattr(inst, "ins", inst),
                                   getattr(g, "ins", g),
                                   sync=True, reason="u after yg")
                return inst

            SCALAR_CHUNKS = {13: 7, 15: 9}  # chunk -> erf idx freeing its slot
            u_ts = {}
            for ci in range(NCH):
                u_t = iop.tile([P, W, D], f32, name="u_t", bufs=U_BUFS)
                u_ts[ci] = u_t
                if ci in SCALAR_CHUNKS:
                    continue  # triggered later, inside the ACT chain
                if ci % 2 == 0:
                    inst = chain(
                        last_sync,
                        nc.sync.dma_start(u_t[:], u_v[:, ci * W:(ci + 1) * W, :]),
                        "syncq")
                    if ci == 0:
                        gate(inst, g_a, g_c)
                else:
                    inst = chain(
                        last_gps,
                        nc.gpsimd.dma_start(u_t[:], u_v[:, ci * W:(ci + 1) * W, :]),
                        "gpsq")
                    if ci == 1:
                        gate(inst, g_a, g_b)

            # --- ACT: exp slices -> ln -> erf chunks (3 table loads total) ---
            # exp runs over SLICES aligned to the yg transfers so it pipelines
            # with their arrival.
            sume = constp.tile([P, RPP], f32)
            exp_slices = [(0, A1), (A1, B1), (B1, RPP)]
            for r0, r1 in exp_slices:
                ex_t = workp.tile([P, r1 - r0, C], bf16,
                                  name=f"ex_{r1 - r0}", bufs=1)
                act_ordered(ex_t[:], yg_all[:, r0:r1, :], FT.Exp)
                nc.vector.reduce_sum(sume[:, r0:r1], ex_t[:], axis=AX.X)
            lse = sume  # ln in place: sume becomes logsumexp
            act_ordered(lse[:], sume[:], FT.Ln)

            # --- streamed erf + work-tile assembly + segment matmuls ---
            ps = psump.tile([C, W_COLS], f32)
            for ci in range(NCH):
                r0, r1 = ci * W, (ci + 1) * W
                work_t = workp.tile([P, W, W_COLS], bf16, name="work_t", bufs=WORK_BUFS)
                act_ordered(
                    work_t[:, :, 0:D], u_ts[ci][:], FT.Erf,
                    bias=erf_bias[:], scale=SQ2,
                )
                # fire any scalar-ring tail u trigger whose slot this erf freed
                for cj, freeing in SCALAR_CHUNKS.items():
                    if freeing == ci:
                        trig = nc.scalar.dma_start(
                            u_ts[cj][:], u_v[:, cj * W:(cj + 1) * W, :])
                        add_dep_helper(getattr(trig, "ins", trig), last_act[0],
                                       sync=True, reason="tail u trig in chain")
                oh_t = workp.tile([P, W, C], bf16, name="oh_t", bufs=WORK_BUFS)
                nc.vector.tensor_tensor(
                    oh_t[:],
                    yh_f[:, r0:r1].broadcast_to([P, W, C]),
                    iota_f[:].broadcast_to([P, W, C]),
                    ALU.is_equal,
                )
                nc.vector.tensor_tensor(
                    work_t[:, :, D:D + C], oh_t[:], yg_all[:, r0:r1, :], ALU.mult
                )
                nc.vector.memset(work_t[:, :, D + C], 1.0)
                nc.vector.tensor_copy(work_t[:, :, D + C + 1], lse[:, r0:r1])
                for g in range(W):
                    nc.tensor.matmul(
                        ps[:], oh_t[:, g, :], work_t[:, g, :],
                        start=(ci == 0 and g == 0),
                        stop=(ci == NCH - 1 and g == W - 1),
                    )

            # --- raw partials out; host finishes the 40-flop epilogue ---
            # out rides the (otherwise idle) scalar HWDGE ring, skipping the
            # sync ring's FIFO tail
            acc = constp.tile([C, W_COLS], f32)
            nc.vector.tensor_copy(acc[:], ps[:])
            out_trig = nc.scalar.dma_start(out_d[:], acc[:])
            add_dep_helper(getattr(out_trig, "ins", out_trig), last_act[0],
                           sync=True, reason="out trigger after last erf")

    nc.compile()
    return nc


_NC_CACHE = {}


def _get_nc():
    if "nc" not in _NC_CACHE:
        _NC_CACHE["nc"] = build()
    return _NC_CACHE["nc"]


def make_in_maps(yhat, yg, u_zg):
    yhat = np.ascontiguousarray(np.asarray(yhat).astype(np.int32))
    yg = np.ascontiguousarray(np.asarray(yg, dtype=np.float32))
    u_zg = np.ascontiguousarray(np.asarray(u_zg, dtype=np.float32))
    n = yhat.shape[0]
    assert n == ROWS_PER_CORE * N_CORES
    in_maps = []
    for i in range(N_CORES):
        s = slice(i * ROWS_PER_CORE, (i + 1) * ROWS_PER_CORE)
        in_maps.append({"yhat": yhat[s], "yg": yg[s], "u_zg": u_zg[s]})
    return in_maps


def _finalize(parts, lmbd):
    """Gather the 8 per-core [10,76] partials and finish in float64.
    cols 0:64 seg erf | 64:74 seg onehot*yg | 74 counts | 75 seg lse."""
    s = np.zeros((C, W_COLS), dtype=np.float64)
    for p in parts:
        s += np.asarray(p, dtype=np.float64)
    seg_erf = s[:, 0:D].sum(axis=1)
    picked = s[:, D:D + C].sum(axis=1)
    cnt = s[:, D + C]
    seg_lse = s[:, D + C + 1]
    present = cnt > 0
    denom = np.where(present, cnt, 1.0)
    # f(u) = 0.5 + 0.5*erf((u+0.5)*sqrt2): seg_f = 0.5*D*cnt + 0.5*seg_erf
    reg_c = (0.5 * D * cnt + 0.5 * seg_erf) / (denom * D)
    aux_c = (seg_lse - picked) / denom
    nuq = present.sum()
    val = (
        np.where(present, aux_c, 0.0).sum()
        + float(np.asarray(lmbd).reshape(())) * np.where(present, reg_c, 0.0).sum()
    ) / nuq
    return np.float32(val)


def run(yhat, yg, u_zg, lmbd, trace=False):
    from concourse import bass_utils

    nc = _get_nc()
    in_maps = make_in_maps(yhat, yg, u_zg)
    res = bass_utils.run_bass_kernel_spmd(
        nc, in_maps, core_ids=list(range(N_CORES)), trace=trace
    )
    parts = [res.results[i]["out"] for i in range(N_CORES)]
    return _finalize(parts, lmbd), res


def kernel(yhat, yg, u_zg, lmbd):
    val, _ = run(yhat, yg, u_zg, lmbd)
    return np.asarray(val, dtype=np.float32).reshape(())
